# revision 17
# baseline (speedup 1.0000x reference)
"""Multi-head attention (RoPE) Trainium2 Bass kernel, v2.

Problem: B=2, S=2048, d_model=1024, 16 heads x head_dim 64, fp32.

Reference replicates a torch rank-5 reshape bug (see kernel_v1): the
output projection is per-head; every (b,h) yields an independent
[128, 1024] output block placed by the host.

v2 restructure vs v1 (297us): the softmax exp on the Scalar/ACT engine
(~137us total) is the true bottleneck, so the schedule maximizes ACT
saturation:
  - startup DMAs reordered (wk + xt first) so K-proj starts early
  - Phase B does only K+V (+rope K) for all tiles, then Q(tile 0);
    remaining Q projections are emitted inside C(hc=0)'s qt loop
  - normalization deferred: AV drains copy unnormalized O^T into ot64
    and DMA the denominator row into a gather tile; ONE batched
    reciprocal per head-pair (v1 spent 52us on 16 single-lane recips)
  - Phase D for head-pair 0 is interleaved into C(hc=1)'s kc loop;
    only D(hc=1) (~10us) remains as tail
  - PSUM: C uses ps_s bufs=2 (4 banks) + u (2) leaving 2 banks for the
    Q-proj (hc=0) / D-chunk (hc=1) rotations
"""

import numpy as np

import concourse.bass as bass
import concourse.tile as tile
from concourse import bacc, mybir
from concourse import bass_utils

F32 = mybir.dt.float32
MM_DT = mybir.dt.float32r  # matmul operand dtype (float32r: 1 cyc/row)

B, S, DM, H, HD = 2, 2048, 1024, 16, 64
N_CORES = 8
HG = 4          # head groups (tensor-parallel factor)
GD = DM // HG   # qkv dims per core = 256
NKC = DM // 128   # d_model contraction chunks = 8
NST = S // 512    # seq tiles of 512 = 4
NSK = S // 128    # seq_k chunks of 128 = 16
LAG = 2           # AV matmuls trail score matmuls by LAG kc iterations


def _emit(nc, tc, ap):
    import contextlib

    ctx = contextlib.ExitStack()
    with ctx:
        consts = ctx.enter_context(tc.tile_pool(name="consts", bufs=1))
        big = ctx.enter_context(tc.tile_pool(name="big", bufs=1))

        # ---- weights/consts to SBUF; wk first (K-proj gates everything) ----
        wk = consts.tile([128, NKC, GD], MM_DT)
        nc.sync.dma_start(wk, ap["wk"].rearrange("(kc p) m -> p kc m", p=128).bitcast(MM_DT))
        wv = consts.tile([128, NKC, GD], MM_DT)
        nc.sync.dma_start(wv, ap["wv"].rearrange("(kc p) m -> p kc m", p=128).bitcast(MM_DT))
        wq = consts.tile([128, NKC, GD], MM_DT)
        nc.sync.dma_start(wq, ap["wq"].rearrange("(kc p) m -> p kc m", p=128).bitcast(MM_DT))
        cosb = consts.tile([128, S], F32)
        nc.gpsimd.dma_start(cosb, ap["cosb"])
        sinb = consts.tile([128, S], F32)
        nc.gpsimd.dma_start(sinb, ap["sinb"])
        rot = consts.tile([128, 128], MM_DT)
        nc.gpsimd.dma_start(rot, ap["rot"].bitcast(MM_DT))
        bqc = consts.tile([128, 2], F32)
        nc.gpsimd.dma_start(bqc, ap["bq2"].rearrange("c p -> p c"))
        bkc = consts.tile([128, 2], F32)
        nc.gpsimd.dma_start(bkc, ap["bk2"].rearrange("c p -> p c"))
        bvb = consts.tile([128, GD], F32)
        nc.gpsimd.dma_start(bvb, ap["bv"].partition_broadcast(128))
        # softmax denominators, gathered per head pair: row = qt*2 + hi
        den = [consts.tile([8, 512], F32, name=f"den{h}") for h in range(2)]
        den_r = [consts.tile([8, 512], F32, name=f"den_r{h}") for h in range(2)]

        # ---- persistent activation buffers ----
        qe = [big.tile([128, S], MM_DT, name=f"qe{mc}", tag=f"qe{mc}") for mc in range(2)]
        ke = [big.tile([128, S], MM_DT, name=f"ke{mc}", tag=f"ke{mc}") for mc in range(2)]
        # V natural layout + ones column: [128 seq, kc, head, 65]
        vsb = big.tile([128, NSK, 4, 65], MM_DT, name="vsb", tag="vsb")
        nc.vector.memset(vsb[:, :, :, 64:65].bitcast(F32), 1.0)
        # attention output O^T, heads on free axis: [128, head, S].
        # Partitions 0-63 hold O^T; 64-127 a duplicated copy for phase D
        # row-group pairing. Unnormalized until the per-hc normalize pass.
        ot64 = big.tile([128, 4, S], MM_DT, name="ot64", tag="ot64")

        xt_pool = ctx.enter_context(tc.tile_pool(name="xt", bufs=6))
        raw_pool = ctx.enter_context(tc.tile_pool(name="raw", bufs=2))
        t1_pool = ctx.enter_context(tc.tile_pool(name="t1", bufs=3))

        def rope_pair(raws, dst, psum_pool, sl, tag):
            """raws: {mc: raw tile}; dst: qe or ke; sl: seq slice."""
            rpss = {}
            for mc in range(2):
                rps = psum_pool.tile([128, 512], F32, name=f"rps{mc}", tag=f"{tag}{mc}")
                nc.tensor.matmul(rps, lhsT=rot, rhs=raws[mc], start=True, stop=True)
                rpss[mc] = rps
            for mc in range(2):
                t1 = t1_pool.tile([128, 512], F32)
                nc.vector.tensor_mul(t1, rpss[mc], sinb[:, sl])
                d = dst[mc][:, sl]
                nc.vector.tensor_mul(d, raws[mc], cosb[:, sl])
                nc.vector.tensor_add(d, d, t1)

        def q_proj_half(st, half, state, psum_pool):
            """Project+rope Q for seq tile st into qe (xt re-loaded).

            Emitted in two halves (kc 0-3, then 4-7 + rope) so the PE burst
            between exp emissions stays ~2us and ACT never starves."""
            sl = slice(st * 512, (st + 1) * 512)
            if half == 0:
                state["pq"] = {
                    mc: psum_pool.tile([128, 512], F32, name=f"pq{mc}", tag=f"aux{mc}")
                    for mc in range(2)
                }
            pq = state["pq"]
            for kc in range(half * 4, half * 4 + 4):
                xt_kc = xt_pool.tile([128, 512], MM_DT)
                nc.gpsimd.dma_start(
                    xt_kc, ap["xt"][kc * 128:(kc + 1) * 128, sl].bitcast(MM_DT)
                )
                for mc in range(2):
                    nc.tensor.matmul(
                        pq[mc],
                        lhsT=wq[:, kc, mc * 128:(mc + 1) * 128],
                        rhs=xt_kc,
                        start=(kc == 0),
                        stop=(kc == NKC - 1),
                    )
            if half == 1:
                raws = {}
                for mc in range(2):
                    raw = raw_pool.tile([128, 512], MM_DT, name=f"rawq{mc}", tag=f"rawq{mc}")
                    nc.vector.tensor_scalar_add(raw, pq[mc], bqc[:, mc:mc + 1])
                    raws[mc] = raw
                rope_pair(raws, qe, psum_pool, sl, "aux")

        # ================= Phase B: K+V projections, rope K, Q(st0) ========
        with nc.named_scope("phaseB"):
            with tc.tile_pool(name="ps_kv", bufs=1, space="PSUM") as ps_kv:
                for st in range(NST):
                    sl = slice(st * 512, (st + 1) * 512)
                    pk = {}
                    for mc in range(2):
                        pk[mc] = ps_kv.tile([128, 512], F32, name=f"pk{mc}", tag=f"pk{mc}")
                    pv = {}
                    for ss in range(4):
                        pv[ss] = ps_kv.tile([128, GD], F32, name=f"pv{ss}", tag=f"v{ss}")
                    for kc in range(NKC):
                        xt_kc = xt_pool.tile([128, 512], MM_DT)
                        nc.scalar.dma_start(
                            xt_kc, ap["xt"][kc * 128:(kc + 1) * 128, sl].bitcast(MM_DT)
                        )
                        for mc in range(2):
                            nc.tensor.matmul(
                                pk[mc],
                                lhsT=wk[:, kc, mc * 128:(mc + 1) * 128],
                                rhs=xt_kc,
                                start=(kc == 0),
                                stop=(kc == NKC - 1),
                            )
                        for ss in range(4):
                            nc.tensor.matmul(
                                pv[ss],
                                lhsT=xt_kc[:, ss * 128:(ss + 1) * 128],
                                rhs=wv[:, kc, :],
                                start=(kc == 0),
                                stop=(kc == NKC - 1),
                            )
                    raws = {}
                    for mc in range(2):
                        raw = raw_pool.tile([128, 512], MM_DT, name=f"rawk{mc}", tag=f"rawk{mc}")
                        nc.vector.tensor_scalar_add(raw, pk[mc], bkc[:, mc:mc + 1])
                        raws[mc] = raw
                    for ss in range(4):
                        nc.vector.tensor_add(
                            vsb[:, st * 4 + ss, :, 0:64],
                            pv[ss].rearrange("p (h d) -> p h d", h=4),
                            bvb.rearrange("p (h d) -> p h d", h=4),
                        )
                    # rope K: rps reuses the v-tags (pv drained above)
                    rope_pair(raws, ke, ps_kv, sl, "v")
                # Q for tile 0 (reuses pk tags)
                with nc.named_scope("q0"):
                    q_proj_tags = None  # noqa: F841
                    pqs = {}
                    for mc in range(2):
                        pqs[mc] = ps_kv.tile([128, 512], F32, name=f"pq{mc}", tag=f"pk{mc}")
                    for kc in range(NKC):
                        xt_kc = xt_pool.tile([128, 512], MM_DT)
                        nc.scalar.dma_start(
                            xt_kc, ap["xt"][kc * 128:(kc + 1) * 128, 0:512].bitcast(MM_DT)
                        )
                        for mc in range(2):
                            nc.tensor.matmul(
                                pqs[mc],
                                lhsT=wq[:, kc, mc * 128:(mc + 1) * 128],
                                rhs=xt_kc,
                                start=(kc == 0),
                                stop=(kc == NKC - 1),
                            )
                    raws = {}
                    for mc in range(2):
                        raw = raw_pool.tile([128, 512], MM_DT, name=f"rawq{mc}", tag=f"rawq{mc}")
                        nc.vector.tensor_scalar_add(raw, pqs[mc], bqc[:, mc:mc + 1])
                        raws[mc] = raw
                    rope_pair(raws, qe, ps_kv, slice(0, 512), "v")

        # ================= Phase C + D =================
        e_pool = ctx.enter_context(tc.tile_pool(name="e", bufs=LAG + 2))
        rcp_pool = ctx.enter_context(tc.tile_pool(name="rcp", bufs=2))
        rdram_pool = ctx.enter_context(tc.tile_pool(name="rdram", bufs=2, space="DRAM"))
        wo_pool = ctx.enter_context(tc.tile_pool(name="wo", bufs=3))
        y_pool = ctx.enter_context(tc.tile_pool(name="y", bufs=3))
        ps_s = ctx.enter_context(tc.tile_pool(name="ps_s", bufs=2, space="PSUM"))
        ps_u = ctx.enter_context(tc.tile_pool(name="ps_u", bufs=1, space="PSUM"))
        ps_aux = ctx.enter_context(tc.tile_pool(name="ps_aux", bufs=1, space="PSUM"))

        def wo_dma(mc):
            wo_mc = wo_pool.tile([128, 8, 128], MM_DT, name="wo_mc", tag="wo_mc")
            nc.sync.dma_start(
                wo_mc,
                ap["wo"][:, mc * 128:(mc + 1) * 128]
                .rearrange("(c p) m -> p c m", p=128)
                .bitcast(MM_DT),
            )
            return wo_mc

        ot_r = ot64.rearrange("p h (s2r g) -> p h s2r g", g=16)

        def d_chunk(hc, mc, wo_mc):
            """Phase D output-projection chunk for head pair hc, Wo cols mc.

            even/odd j chunks alternate PE row groups (0-63 / 64-127) into
            two accumulators (on the aux PSUM tags) -> concurrent pairs."""
            py = {
                p: ps_aux.tile([128, 256], F32, name=f"py{p}", tag=f"aux{p}")
                for p in range(2)
            }
            for j in range(16):
                base = slice((j % 2) * 64, (j % 2) * 64 + 64)
                nc.tensor.matmul(
                    py[j % 2],
                    lhsT=wo_mc[base, j // 2, :],
                    rhs=ot_r[base, 2 * hc:2 * hc + 2, :, j],
                    start=(j < 2),
                    stop=(j >= 14),
                )
            ta = y_pool.tile([128, 256], F32, name="ta", tag="ta")
            nc.vector.tensor_copy(ta, py[0])
            ysb = y_pool.tile([128, 256], F32, name="ysb", tag="ysb")
            nc.vector.tensor_add(ysb, py[1], ta)
            nc.sync.dma_start(
                ap["ypt"][mc * 128:(mc + 1) * 128, hc * 256:(hc + 1) * 256], ysb
            )

        def attn_qt(hc, qt, interleave):
            """scores/exp/AV for (head pair hc, q tile qt).

            interleave: list of (emit_at_kc, fn) extra emissions on the PE
            stream (Q projections for hc=0, D chunks for hc=1)."""
            qsl = slice(qt * 512, (qt + 1) * 512)
            u = [ps_u.tile([65, 512], F32, name=f"u{i}", tag=f"u{i}") for i in range(2)]
            es = {}
            intl = dict(interleave)
            for kc in range(NSK + LAG):
                if kc in intl:
                    intl[kc]()
                if kc >= LAG:
                    ka = kc - LAG
                    for hi in range(2):
                        nc.tensor.matmul(
                            u[hi],
                            lhsT=vsb[:, ka, hc * 2 + hi, :],
                            rhs=es[ka][:, hi * 512:(hi + 1) * 512],
                            start=(ka == 0),
                            stop=(ka == NSK - 1),
                        )
                    if ka > 0:
                        del es[ka - 1]
                if kc < NSK:
                    g = ps_s.tile([128, 1024], F32, tag="sg", name="sg")
                    for hi in range(2):
                        hpart = slice(hi * 64, (hi + 1) * 64)
                        nc.tensor.matmul(
                            g[:, hi * 512:(hi + 1) * 512],
                            lhsT=ke[hc][hpart, kc * 128:(kc + 1) * 128],
                            rhs=qe[hc][hpart, qsl],
                            start=True,
                            stop=True,
                        )
                    e = e_pool.tile([128, 1024], MM_DT, name="e", tag="e")
                    nc.scalar.activation(
                        e, g, mybir.ActivationFunctionType.Exp, scale=0.125
                    )
                    es[kc] = e
            # drain: unnormalized O^T (plus denom row on partition 64, which
            # the dup pass later overwrites) -> ot64; denom row -> den gather
            for hi in range(2):
                idx = qt * 2 + hi
                nc.vector.tensor_copy(ot64[0:65, hc * 2 + hi, qsl], u[hi][0:65, :])
                nc.sync.dma_start(
                    den[hc][idx:idx + 1, :], ot64[64:65, hc * 2 + hi, qsl].bitcast(F32)
                )

        def normalize(hc):
            """Batched reciprocal + in-place normalize + dup for head pair."""
            nc.vector.reciprocal(den_r[hc], den[hc])
            # DRAM bounce: partition-broadcast DMA needs a DRAM source
            rd = rdram_pool.tile([8, 512], F32, tag="rd")
            nc.sync.dma_start(rd, den_r[hc])
            for qt in range(NST):
                qsl = slice(qt * 512, (qt + 1) * 512)
                for hi in range(2):
                    i8 = qt * 2 + hi
                    h = hc * 2 + hi
                    dbc = rcp_pool.tile([64, 512], F32, tag="dbc")
                    nc.gpsimd.dma_start(
                        dbc, rd[i8:i8 + 1, :].partition_broadcast(64)
                    )
                    nc.vector.tensor_mul(
                        ot64[0:64, h, qsl], ot64[0:64, h, qsl], dbc
                    )
                    nc.gpsimd.dma_start(
                        ot64[64:128, h, qsl], ot64[0:64, h, qsl]
                    )

        with nc.named_scope("phaseC0"):
            for qt in range(NST):
                # project Q for the NEXT tile while ACT chews on this one
                intl = []
                if qt < NST - 1:
                    qstate = {}
                    intl = [
                        (6, lambda st=qt + 1, s=qstate: q_proj_half(st, 0, s, ps_aux)),
                        (11, lambda st=qt + 1, s=qstate: q_proj_half(st, 1, s, ps_aux)),
                    ]
                attn_qt(0, qt, intl)
        with nc.named_scope("norm0"):
            normalize(0)

        with nc.named_scope("phaseC1"):
            wo_tiles = {}
            wo_tiles[0] = wo_dma(0)
            dcount = [0]

            def emit_d():
                mc = dcount[0]
                if mc < NKC:
                    if mc + 1 < NKC:
                        wo_tiles[mc + 1] = wo_dma(mc + 1)
                    d_chunk(0, mc, wo_tiles.pop(mc))
                    dcount[0] += 1

            for qt in range(NST):
                intl = [(4, emit_d), (12, emit_d)]
                attn_qt(1, qt, intl)
        with nc.named_scope("norm1"):
            normalize(1)

        with nc.named_scope("phaseD1"):
            wo_tiles = {0: wo_dma(0)}
            for mc in range(NKC):
                if mc + 1 < NKC:
                    wo_tiles[mc + 1] = wo_dma(mc + 1)
                d_chunk(1, mc, wo_tiles.pop(mc))


def _build():
    nc = bacc.Bacc("TRN2", target_bir_lowering=False, debug=False, num_devices=N_CORES)
    ap = {}
    ap["xt"] = nc.dram_tensor("xt", [DM, S], F32, kind="ExternalInput").ap()
    ap["wq"] = nc.dram_tensor("wq", [DM, GD], F32, kind="ExternalInput").ap()
    ap["wk"] = nc.dram_tensor("wk", [DM, GD], F32, kind="ExternalInput").ap()
    ap["wv"] = nc.dram_tensor("wv", [DM, GD], F32, kind="ExternalInput").ap()
    ap["wo"] = nc.dram_tensor("wo", [DM, DM], F32, kind="ExternalInput").ap()
    ap["bq2"] = nc.dram_tensor("bq2", [2, 128], F32, kind="ExternalInput").ap()
    ap["bk2"] = nc.dram_tensor("bk2", [2, 128], F32, kind="ExternalInput").ap()
    ap["bv"] = nc.dram_tensor("bv", [GD], F32, kind="ExternalInput").ap()
    ap["cosb"] = nc.dram_tensor("cosb", [128, S], F32, kind="ExternalInput").ap()
    ap["sinb"] = nc.dram_tensor("sinb", [128, S], F32, kind="ExternalInput").ap()
    ap["rot"] = nc.dram_tensor("rot", [128, 128], F32, kind="ExternalInput").ap()
    # per-core output: Y^T [1024, 512] (columns = 4 heads x 128 block rows)
    ap["ypt"] = nc.dram_tensor("ypt", [DM, 512], F32, kind="ExternalOutput").ap()

    with tile.TileContext(nc) as tc:
        _emit(nc, tc, ap)
    nc.compile()
    return nc


_CACHE = {}


def _rope_tables():
    inv_freq = (1.0 / (10000.0 ** (np.arange(0, HD, 2, dtype=np.float32) / HD))).astype(np.float32)
    t = np.arange(S, dtype=np.float32)
    freqs = np.outer(t, inv_freq).astype(np.float32)  # [S, 32]
    emb = np.concatenate([freqs, freqs], axis=-1)  # [S, 64]
    cosT = np.cos(emb).astype(np.float32).T  # [64, S]
    sinT = np.sin(emb).astype(np.float32).T
    cosb = np.ascontiguousarray(np.concatenate([cosT, cosT], axis=0))  # [128, S]
    sinb = np.ascontiguousarray(np.concatenate([sinT, sinT], axis=0))
    return cosb, sinb


def _rot_matrix():
    p64 = np.zeros((HD, HD), dtype=np.float32)
    for i in range(32):
        p64[i, i + 32] = -1.0
        p64[i + 32, i] = 1.0
    p = np.zeros((128, 128), dtype=np.float32)
    p[0:64, 0:64] = p64
    p[64:128, 64:128] = p64
    return np.ascontiguousarray(p.T)  # lhsT = P^T


def kernel(x, Wq, bq, Wk, bk, Wv, bv, Wo, bo):
    x = np.asarray(x, dtype=np.float32)
    Wq, bq = np.asarray(Wq, np.float32), np.asarray(bq, np.float32)
    Wk, bk = np.asarray(Wk, np.float32), np.asarray(bk, np.float32)
    Wv, bv = np.asarray(Wv, np.float32), np.asarray(bv, np.float32)
    Wo, bo = np.asarray(Wo, np.float32), np.asarray(bo, np.float32)

    if "nc" not in _CACHE:
        _CACHE["nc"] = _build()
    nc = _CACHE["nc"]

    cosb, sinb = _rope_tables()
    rot = _rot_matrix()
    xt_b = [np.ascontiguousarray(x[b].T) for b in range(B)]  # [DM, S]
    wo_c = np.ascontiguousarray(Wo)

    in_maps = []
    for c in range(N_CORES):
        b, hg = divmod(c, HG)
        sl = slice(hg * GD, (hg + 1) * GD)
        in_maps.append(
            {
                "xt": xt_b[b],
                "wq": np.ascontiguousarray(Wq[:, sl]),
                "wk": np.ascontiguousarray(Wk[:, sl]),
                "wv": np.ascontiguousarray(Wv[:, sl]),
                "wo": wo_c,
                "bq2": np.ascontiguousarray(bq[sl].reshape(2, 128)),
                "bk2": np.ascontiguousarray(bk[sl].reshape(2, 128)),
                "bv": np.ascontiguousarray(bv[sl]),
                "cosb": cosb,
                "sinb": sinb,
                "rot": rot,
            }
        )

    res = bass_utils.run_bass_kernel_spmd(nc, in_maps, core_ids=list(range(N_CORES)))
    _CACHE["last_results"] = res

    # Block placement: core (b, hg), local head hl -> global head h = hg*4+hl,
    # lands at out[h//8, (h%8)*256 + b*128 : +128, :].
    out = np.empty((B, S, DM), dtype=np.float32)
    for c in range(N_CORES):
        b, hg = divmod(c, HG)
        ypt = res.results[c]["ypt"]  # [1024, 512]
        for hl in range(4):
            h = hg * 4 + hl
            b2 = h // 8
            s2 = (h % 8) * 256 + b * 128
            out[b2, s2:s2 + 128, :] = ypt[:, hl * 128:(hl + 1) * 128].T
    out += bo[None, None, :]
    return out


# revision 25
# speedup vs baseline: 1.0638x; 1.0638x over previous
"""Multi-head attention (RoPE) Trainium2 Bass kernel, v2.

Problem: B=2, S=2048, d_model=1024, 16 heads x head_dim 64, fp32.

Reference replicates a torch rank-5 reshape bug (see kernel_v1): the
output projection is per-head; every (b,h) yields an independent
[128, 1024] output block placed by the host.

v2 restructure vs v1 (297us): the softmax exp on the Scalar/ACT engine
(~137us total) is the true bottleneck, so the schedule maximizes ACT
saturation:
  - startup DMAs reordered (wk + xt first) so K-proj starts early
  - Phase B does only K+V (+rope K) for all tiles, then Q(tile 0);
    remaining Q projections are emitted inside C(hc=0)'s qt loop
  - normalization deferred: AV drains copy unnormalized O^T into ot64
    and DMA the denominator row into a gather tile; ONE batched
    reciprocal per head-pair (v1 spent 52us on 16 single-lane recips)
  - Phase D for head-pair 0 is interleaved into C(hc=1)'s kc loop;
    only D(hc=1) (~10us) remains as tail
  - PSUM: C uses ps_s bufs=2 (4 banks) + u (2) leaving 2 banks for the
    Q-proj (hc=0) / D-chunk (hc=1) rotations
"""

import numpy as np

import concourse.bass as bass
import concourse.tile as tile
from concourse import bacc, mybir
from concourse import bass_utils

F32 = mybir.dt.float32
MM_DT = mybir.dt.float32r  # matmul operand dtype (float32r: 1 cyc/row)

B, S, DM, H, HD = 2, 2048, 1024, 16, 64
N_CORES = 8
HG = 4          # head groups (tensor-parallel factor)
GD = DM // HG   # qkv dims per core = 256
NKC = DM // 128   # d_model contraction chunks = 8
NST = S // 512    # seq tiles of 512 = 4
NSK = S // 128    # seq_k chunks of 128 = 16
LAG = 2           # AV matmuls trail score matmuls by LAG kc iterations


def _emit(nc, tc, ap):
    import contextlib

    ctx = contextlib.ExitStack()
    with ctx:
        consts = ctx.enter_context(tc.tile_pool(name="consts", bufs=1))
        big = ctx.enter_context(tc.tile_pool(name="big", bufs=1))

        # ---- weights/consts to SBUF; wk first (K-proj gates everything) ----
        wk = consts.tile([128, NKC, GD], MM_DT)
        nc.sync.dma_start(wk, ap["wk"].rearrange("(kc p) m -> p kc m", p=128).bitcast(MM_DT))
        wv = consts.tile([128, NKC, GD], MM_DT)
        nc.sync.dma_start(wv, ap["wv"].rearrange("(kc p) m -> p kc m", p=128).bitcast(MM_DT))
        wq = consts.tile([128, NKC, GD], MM_DT)
        nc.sync.dma_start(wq, ap["wq"].rearrange("(kc p) m -> p kc m", p=128).bitcast(MM_DT))
        cosb = consts.tile([128, S], F32)
        nc.gpsimd.dma_start(cosb, ap["cosb"])
        sinb = consts.tile([128, S], F32)
        nc.gpsimd.dma_start(sinb, ap["sinb"])
        rot = consts.tile([128, 128], MM_DT)
        nc.gpsimd.dma_start(rot, ap["rot"].bitcast(MM_DT))
        bqc = consts.tile([128, 2], F32)
        nc.gpsimd.dma_start(bqc, ap["bq2"].rearrange("c p -> p c"))
        bkc = consts.tile([128, 2], F32)
        nc.gpsimd.dma_start(bkc, ap["bk2"].rearrange("c p -> p c"))
        bvb = consts.tile([128, GD], F32)
        nc.gpsimd.dma_start(bvb, ap["bv"].partition_broadcast(128))
        # softmax denominators, gathered per (hc,qt) reshaped [1,512]->[4,128]
        # per hi so the DVE reciprocal's free size is 128 (cost ~ free size)
        den = {
            (hc, qt): consts.tile([8, 128], F32, name=f"den{hc}{qt}")
            for hc in range(2)
            for qt in range(NST)
        }

        # ---- persistent activation buffers ----
        qe = [big.tile([128, S], MM_DT, name=f"qe{mc}", tag=f"qe{mc}") for mc in range(2)]
        ke = [big.tile([128, S], MM_DT, name=f"ke{mc}", tag=f"ke{mc}") for mc in range(2)]
        # V natural layout + ones column: [128 seq, kc, head, 65]
        vsb = big.tile([128, NSK, 4, 65], MM_DT, name="vsb", tag="vsb")
        nc.vector.memset(vsb[:, :, :, 64:65].bitcast(F32), 1.0)
        # attention output O^T, heads on free axis: [128, head, S].
        # Partitions 0-63 hold O^T; 64-127 a duplicated copy for phase D
        # row-group pairing. Unnormalized until the per-hc normalize pass.
        ot64 = big.tile([128, 4, S], MM_DT, name="ot64", tag="ot64")

        xt_pool = ctx.enter_context(tc.tile_pool(name="xt", bufs=6))
        raw_pool = ctx.enter_context(tc.tile_pool(name="raw", bufs=2))
        t1_pool = ctx.enter_context(tc.tile_pool(name="t1", bufs=3))

        def rope_pair(raws, dst, psum_pool, sl, tag):
            """raws: {mc: raw tile}; dst: qe or ke; sl: seq slice."""
            rpss = {}
            for mc in range(2):
                rps = psum_pool.tile([128, 512], F32, name=f"rps{mc}", tag=f"{tag}{mc}")
                nc.tensor.matmul(rps, lhsT=rot, rhs=raws[mc], start=True, stop=True)
                rpss[mc] = rps
            for mc in range(2):
                t1 = t1_pool.tile([128, 512], F32)
                nc.vector.tensor_mul(t1, rpss[mc], sinb[:, sl])
                d = dst[mc][:, sl]
                nc.vector.tensor_mul(d, raws[mc], cosb[:, sl])
                nc.vector.tensor_add(d, d, t1)

        def q_proj_half(st, half, state, psum_pool):
            """Project+rope Q for seq tile st into qe (xt re-loaded).

            Emitted in two halves (kc 0-3, then 4-7 + rope) so the PE burst
            between exp emissions stays ~2us and ACT never starves."""
            sl = slice(st * 512, (st + 1) * 512)
            if half == 0:
                state["pq"] = {
                    mc: psum_pool.tile([128, 512], F32, name=f"pq{mc}", tag=f"aux{mc}")
                    for mc in range(2)
                }
            pq = state["pq"]
            for kc in range(half * 4, half * 4 + 4):
                xt_kc = xt_pool.tile([128, 512], MM_DT)
                nc.gpsimd.dma_start(
                    xt_kc, ap["xt"][kc * 128:(kc + 1) * 128, sl].bitcast(MM_DT)
                )
                for mc in range(2):
                    nc.tensor.matmul(
                        pq[mc],
                        lhsT=wq[:, kc, mc * 128:(mc + 1) * 128],
                        rhs=xt_kc,
                        start=(kc == 0),
                        stop=(kc == NKC - 1),
                    )
            if half == 1:
                raws = {}
                for mc in range(2):
                    raw = raw_pool.tile([128, 512], MM_DT, name=f"rawq{mc}", tag=f"rawq{mc}")
                    nc.vector.tensor_scalar_add(raw, pq[mc], bqc[:, mc:mc + 1])
                    raws[mc] = raw
                rope_pair(raws, qe, psum_pool, sl, "aux")

        # ================= Phase B: K+V projections, rope K, Q(st0) ========
        with nc.named_scope("phaseB"):
            with tc.tile_pool(name="ps_kv", bufs=1, space="PSUM") as ps_kv:
                for st in range(NST):
                    sl = slice(st * 512, (st + 1) * 512)
                    pk = {}
                    for mc in range(2):
                        pk[mc] = ps_kv.tile([128, 512], F32, name=f"pk{mc}", tag=f"pk{mc}")
                    pv = {}
                    for ss in range(4):
                        pv[ss] = ps_kv.tile([128, GD], F32, name=f"pv{ss}", tag=f"v{ss}")
                    for kc in range(NKC):
                        xt_kc = xt_pool.tile([128, 512], MM_DT)
                        # alternate DMA queues: one ring can't feed the PE
                        eng = nc.scalar if kc % 2 == 0 else nc.sync
                        eng.dma_start(
                            xt_kc, ap["xt"][kc * 128:(kc + 1) * 128, sl].bitcast(MM_DT)
                        )
                        for mc in range(2):
                            nc.tensor.matmul(
                                pk[mc],
                                lhsT=wk[:, kc, mc * 128:(mc + 1) * 128],
                                rhs=xt_kc,
                                start=(kc == 0),
                                stop=(kc == NKC - 1),
                            )
                        for ss in range(4):
                            nc.tensor.matmul(
                                pv[ss],
                                lhsT=xt_kc[:, ss * 128:(ss + 1) * 128],
                                rhs=wv[:, kc, :],
                                start=(kc == 0),
                                stop=(kc == NKC - 1),
                            )
                    raws = {}
                    for mc in range(2):
                        raw = raw_pool.tile([128, 512], MM_DT, name=f"rawk{mc}", tag=f"rawk{mc}")
                        nc.vector.tensor_scalar_add(raw, pk[mc], bkc[:, mc:mc + 1])
                        raws[mc] = raw
                    for ss in range(4):
                        nc.vector.tensor_add(
                            vsb[:, st * 4 + ss, :, 0:64],
                            pv[ss].rearrange("p (h d) -> p h d", h=4),
                            bvb.rearrange("p (h d) -> p h d", h=4),
                        )
                    # rope K: rps reuses the v-tags (pv drained above)
                    rope_pair(raws, ke, ps_kv, sl, "v")
                # Q for tile 0 (reuses pk tags)
                with nc.named_scope("q0"):
                    q_proj_tags = None  # noqa: F841
                    pqs = {}
                    for mc in range(2):
                        pqs[mc] = ps_kv.tile([128, 512], F32, name=f"pq{mc}", tag=f"pk{mc}")
                    for kc in range(NKC):
                        xt_kc = xt_pool.tile([128, 512], MM_DT)
                        eng = nc.scalar if kc % 2 == 0 else nc.sync
                        eng.dma_start(
                            xt_kc, ap["xt"][kc * 128:(kc + 1) * 128, 0:512].bitcast(MM_DT)
                        )
                        for mc in range(2):
                            nc.tensor.matmul(
                                pqs[mc],
                                lhsT=wq[:, kc, mc * 128:(mc + 1) * 128],
                                rhs=xt_kc,
                                start=(kc == 0),
                                stop=(kc == NKC - 1),
                            )
                    raws = {}
                    for mc in range(2):
                        raw = raw_pool.tile([128, 512], MM_DT, name=f"rawq{mc}", tag=f"rawq{mc}")
                        nc.vector.tensor_scalar_add(raw, pqs[mc], bqc[:, mc:mc + 1])
                        raws[mc] = raw
                    rope_pair(raws, qe, ps_kv, slice(0, 512), "v")

        # ================= Phase C + D =================
        e_pool = ctx.enter_context(tc.tile_pool(name="e", bufs=LAG + 2))
        rcp_pool = ctx.enter_context(tc.tile_pool(name="rcp", bufs=2))
        rdram_pool = ctx.enter_context(tc.tile_pool(name="rdram", bufs=2, space="DRAM"))
        wo_pool = ctx.enter_context(tc.tile_pool(name="wo", bufs=3))
        y_pool = ctx.enter_context(tc.tile_pool(name="y", bufs=3))
        ps_s = ctx.enter_context(tc.tile_pool(name="ps_s", bufs=2, space="PSUM"))
        ps_u = ctx.enter_context(tc.tile_pool(name="ps_u", bufs=1, space="PSUM"))
        ps_aux = ctx.enter_context(tc.tile_pool(name="ps_aux", bufs=1, space="PSUM"))

        def wo_dma(mc):
            wo_mc = wo_pool.tile([128, 8, 128], MM_DT, name="wo_mc", tag="wo_mc")
            nc.sync.dma_start(
                wo_mc,
                ap["wo"][:, mc * 128:(mc + 1) * 128]
                .rearrange("(c p) m -> p c m", p=128)
                .bitcast(MM_DT),
            )
            return wo_mc

        ot_r = ot64.rearrange("p h (s2r g) -> p h s2r g", g=16)

        def d_chunk(hc, mc, wo_mc):
            """Phase D output-projection chunk for head pair hc, Wo cols mc.

            even/odd j chunks alternate PE row groups (0-63 / 64-127) into
            two accumulators (on the aux PSUM tags) -> concurrent pairs."""
            py = {
                p: ps_aux.tile([128, 256], F32, name=f"py{p}", tag=f"aux{p}")
                for p in range(2)
            }
            for j in range(16):
                base = slice((j % 2) * 64, (j % 2) * 64 + 64)
                nc.tensor.matmul(
                    py[j % 2],
                    lhsT=wo_mc[base, j // 2, :],
                    rhs=ot_r[base, 2 * hc:2 * hc + 2, :, j],
                    start=(j < 2),
                    stop=(j >= 14),
                )
            ta = y_pool.tile([128, 256], F32, name="ta", tag="ta")
            nc.vector.tensor_copy(ta, py[0])
            ysb = y_pool.tile([128, 256], F32, name="ysb", tag="ysb")
            nc.vector.tensor_add(ysb, py[1], ta)
            nc.sync.dma_start(
                ap["ypt"][mc * 128:(mc + 1) * 128, hc * 256:(hc + 1) * 256], ysb
            )

        def attn_qt(hc, qt, interleave):
            """scores/exp/AV for (head pair hc, q tile qt).

            interleave: list of (emit_at_kc, fn) extra emissions on the PE
            stream (Q projections for hc=0, D chunks for hc=1)."""
            qsl = slice(qt * 512, (qt + 1) * 512)
            u = [ps_u.tile([65, 512], F32, name=f"u{i}", tag=f"u{i}") for i in range(2)]
            es = {}
            intl = dict(interleave)
            for kc in range(NSK + LAG):
                if kc in intl:
                    intl[kc]()
                if kc >= LAG:
                    ka = kc - LAG
                    for hi in range(2):
                        nc.tensor.matmul(
                            u[hi],
                            lhsT=vsb[:, ka, hc * 2 + hi, :],
                            rhs=es[ka][:, hi * 512:(hi + 1) * 512],
                            start=(ka == 0),
                            stop=(ka == NSK - 1),
                        )
                    if ka > 0:
                        del es[ka - 1]
                if kc < NSK:
                    g = ps_s.tile([128, 1024], F32, tag="sg", name="sg")
                    for hi in range(2):
                        hpart = slice(hi * 64, (hi + 1) * 64)
                        nc.tensor.matmul(
                            g[:, hi * 512:(hi + 1) * 512],
                            lhsT=ke[hc][hpart, kc * 128:(kc + 1) * 128],
                            rhs=qe[hc][hpart, qsl],
                            start=True,
                            stop=True,
                        )
                    e = e_pool.tile([128, 1024], MM_DT, name="e", tag="e")
                    nc.scalar.activation(
                        e, g, mybir.ActivationFunctionType.Exp, scale=0.125
                    )
                    es[kc] = e
            # drain: unnormalized O^T (plus denom row on partition 64, which
            # the dup pass later overwrites) -> ot64; denom row -> den gather
            for hi in range(2):
                nc.vector.tensor_copy(ot64[0:65, hc * 2 + hi, qsl], u[hi][0:65, :])
                nc.sync.dma_start(
                    den[hc, qt][hi * 4:hi * 4 + 4, :],
                    ot64[64:65, hc * 2 + hi, qsl].bitcast(F32),
                )
            normalize_qt(hc, qt)

        def normalize_qt(hc, qt):
            """Reciprocal + in-place normalize + dup for one (hc, qt)."""
            qsl = slice(qt * 512, (qt + 1) * 512)
            den_r = rcp_pool.tile([8, 128], F32, tag="den_r")
            nc.vector.reciprocal(den_r, den[hc, qt])
            # DRAM bounce: partition-broadcast DMA needs a DRAM source
            rd = rdram_pool.tile([8, 128], F32, tag="rd")
            nc.sync.dma_start(rd, den_r)
            for hi in range(2):
                h = hc * 2 + hi
                dbc = rcp_pool.tile([64, 512], F32, tag="dbc")
                nc.gpsimd.dma_start(
                    dbc,
                    rd.rearrange("(i s) q -> i (s q)", s=4)[hi:hi + 1, :]
                    .partition_broadcast(64),
                )
                nc.vector.tensor_mul(
                    ot64[0:64, h, qsl], ot64[0:64, h, qsl], dbc
                )
                nc.gpsimd.dma_start(
                    ot64[64:128, h, qsl], ot64[0:64, h, qsl]
                )

        with nc.named_scope("phaseC0"):
            for qt in range(NST):
                # project Q for the NEXT tile while ACT chews on this one
                intl = []
                if qt < NST - 1:
                    qstate = {}
                    intl = [
                        (6, lambda st=qt + 1, s=qstate: q_proj_half(st, 0, s, ps_aux)),
                        (11, lambda st=qt + 1, s=qstate: q_proj_half(st, 1, s, ps_aux)),
                    ]
                attn_qt(0, qt, intl)

        with nc.named_scope("phaseC1"):
            wo_tiles = {}
            wo_tiles[0] = wo_dma(0)
            dcount = [0]

            def emit_d():
                mc = dcount[0]
                if mc < NKC:
                    if mc + 1 < NKC:
                        wo_tiles[mc + 1] = wo_dma(mc + 1)
                    d_chunk(0, mc, wo_tiles.pop(mc))
                    dcount[0] += 1

            for qt in range(NST):
                intl = [(4, emit_d), (12, emit_d)]
                attn_qt(1, qt, intl)

        with nc.named_scope("phaseD1"):
            wo_tiles = {0: wo_dma(0)}
            for mc in range(NKC):
                if mc + 1 < NKC:
                    wo_tiles[mc + 1] = wo_dma(mc + 1)
                d_chunk(1, mc, wo_tiles.pop(mc))


def _build():
    nc = bacc.Bacc("TRN2", target_bir_lowering=False, debug=False, num_devices=N_CORES)
    ap = {}
    ap["xt"] = nc.dram_tensor("xt", [DM, S], F32, kind="ExternalInput").ap()
    ap["wq"] = nc.dram_tensor("wq", [DM, GD], F32, kind="ExternalInput").ap()
    ap["wk"] = nc.dram_tensor("wk", [DM, GD], F32, kind="ExternalInput").ap()
    ap["wv"] = nc.dram_tensor("wv", [DM, GD], F32, kind="ExternalInput").ap()
    ap["wo"] = nc.dram_tensor("wo", [DM, DM], F32, kind="ExternalInput").ap()
    ap["bq2"] = nc.dram_tensor("bq2", [2, 128], F32, kind="ExternalInput").ap()
    ap["bk2"] = nc.dram_tensor("bk2", [2, 128], F32, kind="ExternalInput").ap()
    ap["bv"] = nc.dram_tensor("bv", [GD], F32, kind="ExternalInput").ap()
    ap["cosb"] = nc.dram_tensor("cosb", [128, S], F32, kind="ExternalInput").ap()
    ap["sinb"] = nc.dram_tensor("sinb", [128, S], F32, kind="ExternalInput").ap()
    ap["rot"] = nc.dram_tensor("rot", [128, 128], F32, kind="ExternalInput").ap()
    # per-core output: Y^T [1024, 512] (columns = 4 heads x 128 block rows)
    ap["ypt"] = nc.dram_tensor("ypt", [DM, 512], F32, kind="ExternalOutput").ap()

    with tile.TileContext(nc) as tc:
        _emit(nc, tc, ap)
    nc.compile()
    return nc


_CACHE = {}


def _rope_tables():
    inv_freq = (1.0 / (10000.0 ** (np.arange(0, HD, 2, dtype=np.float32) / HD))).astype(np.float32)
    t = np.arange(S, dtype=np.float32)
    freqs = np.outer(t, inv_freq).astype(np.float32)  # [S, 32]
    emb = np.concatenate([freqs, freqs], axis=-1)  # [S, 64]
    cosT = np.cos(emb).astype(np.float32).T  # [64, S]
    sinT = np.sin(emb).astype(np.float32).T
    cosb = np.ascontiguousarray(np.concatenate([cosT, cosT], axis=0))  # [128, S]
    sinb = np.ascontiguousarray(np.concatenate([sinT, sinT], axis=0))
    return cosb, sinb


def _rot_matrix():
    p64 = np.zeros((HD, HD), dtype=np.float32)
    for i in range(32):
        p64[i, i + 32] = -1.0
        p64[i + 32, i] = 1.0
    p = np.zeros((128, 128), dtype=np.float32)
    p[0:64, 0:64] = p64
    p[64:128, 64:128] = p64
    return np.ascontiguousarray(p.T)  # lhsT = P^T


def kernel(x, Wq, bq, Wk, bk, Wv, bv, Wo, bo):
    x = np.asarray(x, dtype=np.float32)
    Wq, bq = np.asarray(Wq, np.float32), np.asarray(bq, np.float32)
    Wk, bk = np.asarray(Wk, np.float32), np.asarray(bk, np.float32)
    Wv, bv = np.asarray(Wv, np.float32), np.asarray(bv, np.float32)
    Wo, bo = np.asarray(Wo, np.float32), np.asarray(bo, np.float32)

    if "nc" not in _CACHE:
        _CACHE["nc"] = _build()
    nc = _CACHE["nc"]

    cosb, sinb = _rope_tables()
    rot = _rot_matrix()
    xt_b = [np.ascontiguousarray(x[b].T) for b in range(B)]  # [DM, S]
    wo_c = np.ascontiguousarray(Wo)

    in_maps = []
    for c in range(N_CORES):
        b, hg = divmod(c, HG)
        sl = slice(hg * GD, (hg + 1) * GD)
        in_maps.append(
            {
                "xt": xt_b[b],
                "wq": np.ascontiguousarray(Wq[:, sl]),
                "wk": np.ascontiguousarray(Wk[:, sl]),
                "wv": np.ascontiguousarray(Wv[:, sl]),
                "wo": wo_c,
                "bq2": np.ascontiguousarray(bq[sl].reshape(2, 128)),
                "bk2": np.ascontiguousarray(bk[sl].reshape(2, 128)),
                "bv": np.ascontiguousarray(bv[sl]),
                "cosb": cosb,
                "sinb": sinb,
                "rot": rot,
            }
        )

    res = bass_utils.run_bass_kernel_spmd(nc, in_maps, core_ids=list(range(N_CORES)))
    _CACHE["last_results"] = res

    # Block placement: core (b, hg), local head hl -> global head h = hg*4+hl,
    # lands at out[h//8, (h%8)*256 + b*128 : +128, :].
    out = np.empty((B, S, DM), dtype=np.float32)
    for c in range(N_CORES):
        b, hg = divmod(c, HG)
        ypt = res.results[c]["ypt"]  # [1024, 512]
        for hl in range(4):
            h = hg * 4 + hl
            b2 = h // 8
            s2 = (h % 8) * 256 + b * 128
            out[b2, s2:s2 + 128, :] = ypt[:, hl * 128:(hl + 1) * 128].T
    out += bo[None, None, :]
    return out
